# revision 43
# baseline (speedup 1.0000x reference)
"""YOLO-style detection loss on 8 Trainium2 NeuronCores (Bass/Tile).

Data-parallel over batch: each of the 8 cores gets B=2 of the 16 batch
items.  Per core we compute partial sums (per layer: cls numerator,
ciou numerator, npos, dedup'd positive-cell objectness sum s2,
objectness softplus plane sum s1); the host sums partials across cores
(the all-reduce) and applies the global npos normalization.

Perf design (from trace analysis):
- The objectness channel is 1 float every 340B, so a sparse strided
  read is packet-bound: ~20.6ns/packet on 16 DMA engines = 63us/core.
  Instead we read QUADS: one 1024B packet covers 4 obj values (256
  contiguous floats); 12.9MB over 12.6K packets ~= 36-40us, split
  across both hardware-DGE rings (sync + scalar).
- obj_t scatter-max is computed on-chip: cell indices are transposed
  via PE, broadcast via K=1 outer-product matmuls, and duplicate
  (same-cell) positives are masked with a strictly-lower-triangular
  compare; s2 is then a masked dot with the gathered obj values.
  This removes the DRAM scatter/readback round trip.
- All transcendentals use the single Softplus activation table (one
  ACT table load for the whole kernel).
"""
import sys
import types

sys.path.insert(0, "/opt/trn_rl_repo")

import numpy as np

import concourse.bacc as bacc
import concourse.bass as bass
import concourse.mybir as mybir
import concourse.tile as tile
from concourse.bass_utils import run_bass_kernel_spmd
from concourse.tile_rust import add_dep_helper

F32 = mybir.dt.float32
I32 = mybir.dt.int32
OP = mybir.AluOpType
AF = mybir.ActivationFunctionType

N_CORES = 8
B_GLOB = 16
B_LOC = B_GLOB // N_CORES          # 2
M = 64                             # boxes per batch item
P = B_LOC * M                      # 128 partitions = (b, m)
NC_CLS = 80
GWS = [80, 40, 20]                 # grid sizes per layer (square)
GHWS = [g * g for g in GWS]        # 6400, 1600, 400
CELLS = [B_LOC * 3 * g * g for g in GWS]   # 38400, 9600, 2400
# plane layouts: (partitions, cells per partition); cells/part % QUAD == 0
PLANE_SHAPES = [(128, 300), (100, 96), (100, 24)]
QUAD = 3
QL = 85 * (QUAD - 1) + 1           # 171 floats = 684B per packet
CLS_GAIN, OBJ_GAIN, BBOX_GAIN = 0.5, 1.0, 0.05
IOU_THR = 0.5

# plane DMA chunks: (layer, quad_start, quad_end); all on the sync ring
# (one hw queue saturates the 16 DMA engines; the scalar engine stays
# free for ACT).  Each chunk owns its tiles so its Abs/Exp/poly/reduce
# pipeline starts as soon as its own DMA lands.
CHUNKS = [
    (2, 0, 8, "sync"),
    (1, 0, 16, "scalar"),
    (1, 16, 32, "scalar"),
    (0, 0, 45, "sync"),
    (0, 45, 85, "sync"),
    (0, 85, 97, "sync"),
    (0, 97, 100, "sync"),
]
# final tile column layout:
# [cls(3), box(3), npos(3), s2(3)], then per chunk i: sum|x| at 12+3i,
# sum x at 13+3i, sum P(e^-|x|) at 14+3i.  The [P, FINAL_COLS] tile is
# written out unreduced; the host sums partitions and cores.
# softplus = (x+|x|)/2 + ln1p(e^-|x|); ln1p(t) ~= P(t)+C0 on [0,1]
FINAL_COLS = 40
# degree-3 chebyshev fit of ln1p on [0,1]; C0 is added host-side
LN1P_C = (9.25032111e-04, 9.79753413e-01, -3.93535802e-01,
          1.06684733e-01)

# packed consts column layout
C_IOTA, C_SCALES, C_GW1, C_GHW, C_B3, C_TRIL, C_IDENT = (
    0, 80, 83, 86, 89, 90, 218)
NCONST = 346
# packed per-core inputs column layout: bx(4), lab(1), val(1), anc(18)
NPACK = 24


def _install_profile_hook():
    """The agent image's antenv lacks axon_hooks; register it so
    run_bass_kernel_spmd(trace=True) can produce NTFF profiles."""
    if "antenv.axon_hooks" in sys.modules:
        return
    hooks = types.ModuleType("antenv.axon_hooks")
    hooks._hook = None

    def _set(h):
        hooks._hook = h

    def _get():
        return hooks._hook

    hooks.set_axon_ntff_profile_hook = _set
    hooks.get_axon_ntff_profile_hook = _get
    sys.modules["antenv.axon_hooks"] = hooks
    import antenv

    antenv.axon_hooks = hooks
    try:
        from trn_agent_boot.trn_boot import _ntff_profile_via_ctypes

        _set(_ntff_profile_via_ctypes("/opt/axon/libaxon_pjrt.so"))
    except Exception:
        pass


def _consts():
    """Per-core constant input tensor [P, NCONST] (same on every core)."""
    c = np.zeros((P, NCONST), np.float32)
    c[:, C_IOTA:C_IOTA + 80] = np.arange(NC_CLS, dtype=np.float32)
    c[:, C_SCALES:C_SCALES + 3] = np.array(GWS, np.float32)
    c[:, C_GW1:C_GW1 + 3] = np.array([g - 1 for g in GWS], np.float32)
    c[:, C_GHW:C_GHW + 3] = np.array(GHWS, np.float32)
    c[:, C_B3] = 3.0 * (np.arange(P) // M)
    c[:, C_TRIL:C_TRIL + P] = np.tril(np.ones((P, P), np.float32), -1)
    c[:, C_IDENT:C_IDENT + P] = np.eye(P, dtype=np.float32)
    return c


def _ap(a, offset, pattern):
    return bass.AP(tensor=a.tensor, offset=offset, ap=pattern)


def build_nc(stage=99.0):
    nc = bacc.Bacc("TRN2", target_bir_lowering=False)

    pred_ext = [
        nc.dram_tensor("p3", [B_LOC, 3, 80, 80, 85], F32, kind="ExternalInput"),
        nc.dram_tensor("p4", [B_LOC, 3, 40, 40, 85], F32, kind="ExternalInput"),
        nc.dram_tensor("p5", [B_LOC, 3, 20, 20, 85], F32, kind="ExternalInput"),
    ]
    pk_ext = nc.dram_tensor("pk", [P, NPACK], F32, kind="ExternalInput")
    cst_ext = nc.dram_tensor("cst", [P, NCONST], F32, kind="ExternalInput")
    out_ext = nc.dram_tensor("out", [P, FINAL_COLS], F32, kind="ExternalOutput")

    with tile.TileContext(nc) as tc:
        with (
            tc.tile_pool(name="sb", bufs=1) as sb,
            tc.tile_pool(name="ps", bufs=1, space="PSUM") as ps,
        ):
            # ---- plane quad loads: one DMA per chunk; most on the sync
            # ring, a small prefix on scalar (its push finishes before the
            # first ACT work needs the scalar engine) ----
            x2c = []
            for i, (li, qa, qb, ring) in enumerate(CHUNKS):
                prt, cpp = PLANE_SHAPES[li]
                w = qb - qa
                t = sb.tile([prt, w * QL], F32, tag=f"x2c{i}",
                            name=f"x2c{i}")
                eng = nc.sync if ring == "sync" else nc.scalar
                eng.dma_start(
                    t[:],
                    _ap(pred_ext[li][:], 4 + 85 * QUAD * qa,
                        [[85 * cpp, prt], [85 * QUAD, w], [1, QL]]))
                x2c.append(t)

            # ---- packed consts + inputs on the gpsimd (software) ring ----
            cst = sb.tile([P, NCONST], F32, tag="cst", name="cst")
            nc.gpsimd.dma_start(cst[:], cst_ext[:])
            pk = sb.tile([P, NPACK], F32, tag="pk", name="pk")
            nc.gpsimd.dma_start(pk[:], pk_ext[:])

            iota80 = cst[:, C_IOTA:C_IOTA + 80]
            scales = cst[:, C_SCALES:C_SCALES + 3]
            gw1 = cst[:, C_GW1:C_GW1 + 3]
            ghw = cst[:, C_GHW:C_GHW + 3]
            b3 = cst[:, C_B3:C_B3 + 1]
            tril = cst[:, C_TRIL:C_TRIL + P]
            ident = cst[:, C_IDENT:C_IDENT + P]
            bx = pk[:, 0:4]
            lab_f = pk[:, 4:5]
            val_f = pk[:, 5:6]
            anc = pk[:, 6:6 + 18]

            # ---- final accumulator tile ----
            final = sb.tile([P, FINAL_COLS], F32, tag="final", name="final")
            nc.vector.memset(final[:], 0.0)
            ones_r = sb.tile([1, P], F32, tag="ones_r", name="ones_r")
            nc.vector.memset(ones_r[:], 1.0)
            c_cls = final[:, 0:3]
            c_box = final[:, 3:6]
            c_npos = final[:, 6:9]
            c_s2 = final[:, 9:12]

            def t3(tag):
                return sb.tile([P, 3], F32, tag=tag, name=tag)

            tt = nc.vector.tensor_tensor
            ts = nc.vector.tensor_scalar
            stt = nc.vector.scalar_tensor_tensor

            # ---- tbox in grid units: cx,cy,w,h [P,3] (col = layer) ----
            s02, s13, d20, d31 = t3("s02"), t3("s13"), t3("d20"), t3("d31")
            tt(out=s02[:, 0:1], in0=bx[:, 0:1], in1=bx[:, 2:3], op=OP.add)
            tt(out=s13[:, 0:1], in0=bx[:, 1:2], in1=bx[:, 3:4], op=OP.add)
            tt(out=d20[:, 0:1], in0=bx[:, 2:3], in1=bx[:, 0:1], op=OP.subtract)
            tt(out=d31[:, 0:1], in0=bx[:, 3:4], in1=bx[:, 1:2], op=OP.subtract)
            cx, cy, w_, h_ = t3("cx"), t3("cy"), t3("w_"), t3("h_")
            stt(out=cx[:], in0=s02[:, 0:1].to_broadcast([P, 3]), scalar=0.5,
                in1=scales, op0=OP.mult, op1=OP.mult)
            stt(out=cy[:], in0=s13[:, 0:1].to_broadcast([P, 3]), scalar=0.5,
                in1=scales, op0=OP.mult, op1=OP.mult)
            tt(out=w_[:], in0=d20[:, 0:1].to_broadcast([P, 3]), in1=scales,
               op=OP.mult)
            tt(out=h_[:], in0=d31[:, 0:1].to_broadcast([P, 3]), in1=scales,
               op=OP.mult)

            # ---- wh-IoU vs anchors: [P, a(3), l(3)] ----
            def rep_a(ap3):  # [P,3] -> [P,3,3] repeating along anchor dim
                return _ap(ap3, ap3.offset, [ap3.ap[0], [0, 3], [1, 3]])

            # anchor (a,l) views into pk: elem 6 + (l*3+a)*2 (+1 for h)
            pkv = pk[:]
            aw9 = _ap(pkv, pkv.offset + 6, [pkv.ap[0], [2, 3], [6, 3]])
            ah9 = _ap(pkv, pkv.offset + 7, [pkv.ap[0], [2, 3], [6, 3]])

            def t33(tag):
                return sb.tile([P, 3, 3], F32, tag=tag, name=tag)

            m1, m2, inter9, u9 = t33("m1"), t33("m2"), t33("inter9"), t33("u9")
            wh3 = t3("wh3")
            tt(out=m1[:], in0=rep_a(w_[:]), in1=aw9, op=OP.min)
            tt(out=m2[:], in0=rep_a(h_[:]), in1=ah9, op=OP.min)
            tt(out=inter9[:], in0=m1[:], in1=m2[:], op=OP.mult)
            tt(out=wh3[:], in0=w_[:], in1=h_[:], op=OP.mult)
            tt(out=u9[:], in0=aw9, in1=ah9, op=OP.mult)
            tt(out=u9[:], in0=u9[:], in1=rep_a(wh3[:]), op=OP.add)
            tt(out=u9[:], in0=u9[:], in1=inter9[:], op=OP.subtract)
            ts(out=u9[:], in0=u9[:], scalar1=1e-6, scalar2=None, op0=OP.add)
            nc.vector.reciprocal(m1[:], u9[:])
            tt(out=inter9[:], in0=inter9[:], in1=m1[:], op=OP.mult)  # iou

            # argmax over anchors (first-max wins, strict >)
            gt1, gt2, b01 = t3("gt1"), t3("gt2"), t3("b01")
            tt(out=gt1[:], in0=inter9[:, 1, :], in1=inter9[:, 0, :], op=OP.is_gt)
            tt(out=b01[:], in0=inter9[:, 0, :], in1=inter9[:, 1, :], op=OP.max)
            tt(out=gt2[:], in0=inter9[:, 2, :], in1=b01[:], op=OP.is_gt)
            tt(out=b01[:], in0=b01[:], in1=inter9[:, 2, :], op=OP.max)  # best
            # pos mask -> final npos cols
            thr = t3("thr")
            ts(out=thr[:], in0=b01[:], scalar1=IOU_THR, scalar2=None,
               op0=OP.is_gt)
            tt(out=c_npos, in0=thr[:], in1=val_f.to_broadcast([P, 3]),
               op=OP.mult)
            m_all = c_npos  # [P,3] mask, also the npos partials
            # a_f = a01 + gt2*(2 - a01)
            a_f, tmp3 = t3("a_f"), t3("tmp3")
            ts(out=tmp3[:], in0=gt1[:], scalar1=-1.0, scalar2=2.0,
               op0=OP.mult, op1=OP.add)
            tt(out=tmp3[:], in0=tmp3[:], in1=gt2[:], op=OP.mult)
            tt(out=a_f[:], in0=gt1[:], in1=tmp3[:], op=OP.add)

            # ---- grid coords: gx = clip(trunc(cx), 0, gw-1) ----
            gxi = sb.tile([P, 3], I32, tag="gxi", name="gxi")
            gyi = sb.tile([P, 3], I32, tag="gyi", name="gyi")
            gx, gy = t3("gx"), t3("gy")
            corr = t3("corr")
            nc.vector.tensor_copy(gxi[:], cx[:])
            nc.vector.tensor_copy(gx[:], gxi[:])
            tt(out=corr[:], in0=gx[:], in1=cx[:], op=OP.is_gt)
            tt(out=gx[:], in0=gx[:], in1=corr[:], op=OP.subtract)
            nc.vector.tensor_copy(gyi[:], cy[:])
            nc.vector.tensor_copy(gy[:], gyi[:])
            tt(out=corr[:], in0=gy[:], in1=cy[:], op=OP.is_gt)
            tt(out=gy[:], in0=gy[:], in1=corr[:], op=OP.subtract)
            ts(out=gx[:], in0=gx[:], scalar1=0.0, scalar2=None, op0=OP.max)
            tt(out=gx[:], in0=gx[:], in1=gw1, op=OP.min)
            ts(out=gy[:], in0=gy[:], scalar1=0.0, scalar2=None, op0=OP.max)
            tt(out=gy[:], in0=gy[:], in1=gw1, op=OP.min)

            # ---- flat cell index: (3b + a)*ghw + gy*gw + gx ----
            cell = t3("cell")
            stt(out=cell[:], in0=a_f[:], scalar=b3, in1=ghw,
                op0=OP.add, op1=OP.mult)
            tmp_b = t3("tmp_b")
            tt(out=tmp_b[:], in0=gy[:], in1=scales, op=OP.mult)
            tt(out=cell[:], in0=cell[:], in1=tmp_b[:], op=OP.add)
            tt(out=cell[:], in0=cell[:], in1=gx[:], op=OP.add)
            idx = sb.tile([P, 3], I32, tag="idx", name="idx")
            nc.vector.tensor_copy(idx[:], cell[:])

            def poly_ln1p(yt, at, et, last_c0=True):
                # yt/at: ping-pong scratch views, et: t = exp(-|x|)
                # P(t) = ((c3 t + c2) t + c1) t   (c0 added host-side)
                c0, c1, c2, c3 = LN1P_C
                ts(out=at, in0=et, scalar1=c3, scalar2=c2,
                   op0=OP.mult, op1=OP.add)
                tt(out=yt, in0=at, in1=et, op=OP.mult)
                stt(out=at, in0=yt, scalar=c1, in1=et,
                    op0=OP.add, op1=OP.mult)
                if last_c0:
                    ts(out=yt, in0=at, scalar1=c0, scalar2=None, op0=OP.add)
                    return yt
                return at

            def process_chunk(i):
                # sum|x|, sum x, sum P(exp(-|x|)) for plane chunk i
                li, qa, qb, ring = CHUNKS[i]
                prt = PLANE_SHAPES[li][0]
                w = qb - qa
                xv = x2c[i][:]
                sv = _ap(xv, xv.offset, [xv.ap[0], [85, QUAD], [QL, w]])
                at = sb.tile([prt, QUAD, w], F32, tag=f"at{i}", name=f"at{i}")
                et = sb.tile([prt, QUAD, w], F32, tag=f"et{i}", name=f"et{i}")
                yt = sb.tile([prt, QUAD, w], F32, tag=f"yt{i}", name=f"yt{i}")
                nc.scalar.activation(at[:], sv, AF.Abs)
                nc.vector.tensor_reduce(
                    out=final[0:prt, 12 + 3 * i:13 + 3 * i], in_=at[:],
                    axis=mybir.AxisListType.XY, op=OP.add)
                nc.vector.tensor_reduce(
                    out=final[0:prt, 13 + 3 * i:14 + 3 * i], in_=sv,
                    axis=mybir.AxisListType.XY, op=OP.add)
                nc.scalar.activation(et[:], at[:], AF.Exp, scale=-1.0)
                yout = poly_ln1p(yt[:], at[:], et[:], last_c0=False)
                nc.vector.tensor_reduce(
                    out=final[0:prt, 14 + 3 * i:15 + 3 * i], in_=yout,
                    axis=mybir.AxisListType.XY, op=OP.add)

            # first chunks land before the gather-dependent cls work
            process_chunk(0)
            process_chunk(1)
            process_chunk(2)

            # ---- gather pred rows at assigned cells: [P, 3*85] ----
            gath = sb.tile([P, 3 * 85], F32, tag="gath", name="gath")
            for li in range(3):
                flat = pred_ext[li][:].rearrange("b a h w c -> (b a h w) c")
                nc.gpsimd.indirect_dma_start(
                    out=gath[:, 85 * li:85 * (li + 1)],
                    out_offset=None,
                    in_=flat,
                    in_offset=bass.IndirectOffsetOnAxis(
                        ap=idx[:, li:li + 1], axis=0),
                )
            gv = gath[:]

            # ---- cls loss: sum_c softplus(x_c) - x_label, masked ----
            # softplus = (x + |x|)/2 + ln1p(exp(-|x|)); ln1p via a DVE
            # polynomial so the whole kernel uses one ACT table (abs/exp).
            sp_abs, sp_x, sp_ln = t3("sp_abs"), t3("sp_x"), t3("sp_ln")
            xlab = t3("xlab")
            ca240 = sb.tile([P, 3 * NC_CLS], F32, tag="ca240", name="ca240")
            ce240 = sb.tile([P, 3 * NC_CLS], F32, tag="ce240", name="ce240")
            cy240 = sb.tile([P, 3 * NC_CLS], F32, tag="cy240", name="cy240")
            for li in range(3):
                nc.scalar.activation(
                    ca240[:, 80 * li:80 * (li + 1)],
                    gath[:, 85 * li + 5:85 * li + 85], AF.Abs)
                nc.vector.tensor_reduce(
                    out=sp_abs[:, li:li + 1],
                    in_=ca240[:, 80 * li:80 * (li + 1)],
                    axis=mybir.AxisListType.X, op=OP.add)
                nc.vector.tensor_reduce(
                    out=sp_x[:, li:li + 1],
                    in_=gath[:, 85 * li + 5:85 * li + 85],
                    axis=mybir.AxisListType.X, op=OP.add)
            nc.scalar.activation(ce240[:], ca240[:], AF.Exp, scale=-1.0)
            clsy = poly_ln1p(cy240[:], ca240[:], ce240[:], last_c0=True)
            for li in range(3):
                nc.vector.tensor_reduce(
                    out=sp_ln[:, li:li + 1],
                    in_=_ap(clsy, clsy.offset + 80 * li,
                            [clsy.ap[0], [1, 80]]),
                    axis=mybir.AxisListType.X, op=OP.add)
            oh = sb.tile([P, NC_CLS], F32, tag="oh", name="oh")
            stt(out=oh[:], in0=iota80, scalar=lab_f, in1=iota80,
                op0=OP.is_equal, op1=OP.bypass)
            oc = sb.tile([P, NC_CLS], F32, tag="oc", name="oc")
            for li in range(3):
                tt(out=oc[:], in0=oh[:],
                   in1=gath[:, 85 * li + 5:85 * li + 85], op=OP.mult)
                nc.vector.tensor_reduce(
                    out=xlab[:, li:li + 1], in_=oc[:],
                    axis=mybir.AxisListType.X, op=OP.add)
            spsum = sp_abs  # reuse: 0.5*(sum|x| + sum x) + sum ln1p
            tt(out=spsum[:], in0=sp_abs[:], in1=sp_x[:], op=OP.add)
            ts(out=spsum[:], in0=spsum[:], scalar1=0.5, scalar2=None,
               op0=OP.mult)
            tt(out=spsum[:], in0=spsum[:], in1=sp_ln[:], op=OP.add)
            tt(out=spsum[:], in0=spsum[:], in1=xlab[:], op=OP.subtract)
            tt(out=c_cls, in0=spsum[:], in1=m_all, op=OP.mult)

            # ---- CIoU ----
            def gcol(cidx):  # [P,3] view of gathered column cidx
                return _ap(gv, gv.offset + cidx, [gv.ap[0], [85, 3]])

            pcx, pcy, pw, ph = gcol(0), gcol(1), gcol(2), gcol(3)
            pw2, ph2 = t3("pw2"), t3("ph2")
            ts(out=pw2[:], in0=pw, scalar1=0.5, scalar2=None, op0=OP.mult)
            ts(out=ph2[:], in0=ph, scalar1=0.5, scalar2=None, op0=OP.mult)
            px1, px2, py1, py2 = t3("px1"), t3("px2"), t3("py1"), t3("py2")
            tt(out=px1[:], in0=pcx, in1=pw2[:], op=OP.subtract)
            tt(out=px2[:], in0=pcx, in1=pw2[:], op=OP.add)
            tt(out=py1[:], in0=pcy, in1=ph2[:], op=OP.subtract)
            tt(out=py2[:], in0=pcy, in1=ph2[:], op=OP.add)
            tw2, th2 = pw2, ph2  # reuse tiles
            ts(out=tw2[:], in0=w_[:], scalar1=0.5, scalar2=None, op0=OP.mult)
            ts(out=th2[:], in0=h_[:], scalar1=0.5, scalar2=None, op0=OP.mult)
            tx1, tx2, ty1, ty2 = t3("tx1"), t3("tx2"), t3("ty1"), t3("ty2")
            tt(out=tx1[:], in0=cx[:], in1=tw2[:], op=OP.subtract)
            tt(out=tx2[:], in0=cx[:], in1=tw2[:], op=OP.add)
            tt(out=ty1[:], in0=cy[:], in1=th2[:], op=OP.subtract)
            tt(out=ty2[:], in0=cy[:], in1=th2[:], op=OP.add)
            ix1, iy1, ix2, iy2 = t3("ix1"), t3("iy1"), t3("ix2"), t3("iy2")
            tt(out=ix1[:], in0=px1[:], in1=tx1[:], op=OP.max)
            tt(out=iy1[:], in0=py1[:], in1=ty1[:], op=OP.max)
            tt(out=ix2[:], in0=px2[:], in1=tx2[:], op=OP.min)
            tt(out=iy2[:], in0=py2[:], in1=ty2[:], op=OP.min)
            iw, ih = t3("iw"), t3("ih")
            tt(out=iw[:], in0=ix2[:], in1=ix1[:], op=OP.subtract)
            ts(out=iw[:], in0=iw[:], scalar1=0.0, scalar2=None, op0=OP.max)
            tt(out=ih[:], in0=iy2[:], in1=iy1[:], op=OP.subtract)
            ts(out=ih[:], in0=ih[:], scalar1=0.0, scalar2=None, op0=OP.max)
            inter = t3("inter")
            tt(out=inter[:], in0=iw[:], in1=ih[:], op=OP.mult)
            a1, a2, au = t3("a1"), t3("a2"), t3("au")
            tt(out=a1[:], in0=px2[:], in1=px1[:], op=OP.subtract)
            tt(out=au[:], in0=py2[:], in1=py1[:], op=OP.subtract)
            tt(out=a1[:], in0=a1[:], in1=au[:], op=OP.mult)
            tt(out=a2[:], in0=tx2[:], in1=tx1[:], op=OP.subtract)
            tt(out=au[:], in0=ty2[:], in1=ty1[:], op=OP.subtract)
            tt(out=a2[:], in0=a2[:], in1=au[:], op=OP.mult)
            tt(out=au[:], in0=a1[:], in1=a2[:], op=OP.add)
            tt(out=au[:], in0=au[:], in1=inter[:], op=OP.subtract)
            ts(out=au[:], in0=au[:], scalar1=1e-7, scalar2=None, op0=OP.add)
            iou = a1  # reuse
            rcp = t3("rcp")
            nc.vector.reciprocal(rcp[:], au[:])
            tt(out=iou[:], in0=inter[:], in1=rcp[:], op=OP.mult)
            # center distance
            ccx, ccy = t3("ccx"), t3("ccy")
            tt(out=ccx[:], in0=px1[:], in1=px2[:], op=OP.add)
            ts(out=ccx[:], in0=ccx[:], scalar1=0.5, scalar2=None, op0=OP.mult)
            tt(out=ccy[:], in0=tx1[:], in1=tx2[:], op=OP.add)
            ts(out=ccy[:], in0=ccy[:], scalar1=0.5, scalar2=None, op0=OP.mult)
            tt(out=ccx[:], in0=ccx[:], in1=ccy[:], op=OP.subtract)
            tt(out=ccx[:], in0=ccx[:], in1=ccx[:], op=OP.mult)  # dx^2
            cd = a2  # reuse
            tt(out=cd[:], in0=py1[:], in1=py2[:], op=OP.add)
            ts(out=cd[:], in0=cd[:], scalar1=0.5, scalar2=None, op0=OP.mult)
            tt(out=ccy[:], in0=ty1[:], in1=ty2[:], op=OP.add)
            ts(out=ccy[:], in0=ccy[:], scalar1=0.5, scalar2=None, op0=OP.mult)
            tt(out=cd[:], in0=cd[:], in1=ccy[:], op=OP.subtract)
            tt(out=cd[:], in0=cd[:], in1=cd[:], op=OP.mult)     # dy^2
            tt(out=cd[:], in0=ccx[:], in1=cd[:], op=OP.add)
            # enclosing box diag
            ex1, ex2 = t3("ex1"), t3("ex2")
            tt(out=ex1[:], in0=px1[:], in1=tx1[:], op=OP.min)
            tt(out=ex2[:], in0=px2[:], in1=tx2[:], op=OP.max)
            tt(out=ex2[:], in0=ex2[:], in1=ex1[:], op=OP.subtract)
            tt(out=ex2[:], in0=ex2[:], in1=ex2[:], op=OP.mult)  # dx^2
            ey1, ey2 = ix1, ix2  # reuse
            tt(out=ey1[:], in0=py1[:], in1=ty1[:], op=OP.min)
            tt(out=ey2[:], in0=py2[:], in1=ty2[:], op=OP.max)
            tt(out=ey2[:], in0=ey2[:], in1=ey1[:], op=OP.subtract)
            tt(out=ey2[:], in0=ey2[:], in1=ey2[:], op=OP.mult)  # dy^2
            dd = ex2
            tt(out=dd[:], in0=ex2[:], in1=ey2[:], op=OP.add)
            ts(out=dd[:], in0=dd[:], scalar1=1e-7, scalar2=None, op0=OP.add)
            nc.vector.reciprocal(rcp[:], dd[:])
            tt(out=cd[:], in0=cd[:], in1=rcp[:], op=OP.mult)
            tt(out=iou[:], in0=iou[:], in1=cd[:], op=OP.subtract)
            ts(out=iou[:], in0=iou[:], scalar1=-1.0, scalar2=1.0,
               op0=OP.mult, op1=OP.add)                          # ciou loss
            tt(out=c_box, in0=iou[:], in1=m_all, op=OP.mult)

            # ---- s2 via on-chip dedup of same-cell positives ----
            # scatter index: valid -> cell, invalid -> dump cell (=6*ghw)
            cm = t3("cm")
            ts(out=cm[:], in0=ghw, scalar1=6.0, scalar2=None, op0=OP.mult)
            sc = t3("sc")
            tt(out=sc[:], in0=cell[:], in1=cm[:], op=OP.subtract)
            tt(out=sc[:], in0=sc[:], in1=m_all, op=OP.mult)
            tt(out=sc[:], in0=sc[:], in1=cm[:], op=OP.add)
            # transpose each layer's cell column via PE: scT_l[1, P]
            scT_ps = ps.tile([1, 3 * P], F32, tag="scT_ps", name="scT_ps")
            scT = sb.tile([1, 3 * P], F32, tag="scT", name="scT")
            for li in range(3):
                nc.tensor.matmul(out=scT_ps[:, li * P:(li + 1) * P],
                                 lhsT=sc[:, li:li + 1], rhs=ident,
                                 start=True, stop=True)
            nc.vector.tensor_copy(scT[:], scT_ps[:])
            # broadcast each row across partitions via K=1 outer product
            bc_ps = ps.tile([P, 3 * P], F32, tag="bc_ps", name="bc_ps")
            for li in range(3):
                nc.tensor.matmul(out=bc_ps[:, li * P:(li + 1) * P],
                                 lhsT=ones_r[:], rhs=scT[:, li * P:(li + 1) * P],
                                 start=True, stop=True)
            # dup[p] = any earlier same-batch positive with equal cell
            eq = sb.tile([P, P], F32, tag="eq", name="eq")
            dup3 = t3("dup3")
            for li in range(3):
                stt(out=eq[:], in0=bc_ps[:, li * P:(li + 1) * P],
                    scalar=sc[:, li:li + 1], in1=tril,
                    op0=OP.is_equal, op1=OP.mult)
                nc.vector.tensor_reduce(
                    out=dup3[:, li:li + 1], in_=eq[:],
                    axis=mybir.AxisListType.X, op=OP.max)
            keep = t3("keep")
            ts(out=keep[:], in0=dup3[:], scalar1=-1.0, scalar2=1.0,
               op0=OP.mult, op1=OP.add)
            tt(out=keep[:], in0=keep[:], in1=m_all, op=OP.mult)
            tt(out=c_s2, in0=keep[:], in1=gcol(4), op=OP.mult)

            # ---- remaining obj plane chunks (land later in the drain) ----
            for i in range(3, len(CHUNKS)):
                process_chunk(i)

            # ---- write out unreduced; host sums partitions + cores ----
            nc.scalar.dma_start(out_ext[:], final[:])

    nc.finalize()
    return nc


_NC = None


def _get_nc():
    global _NC
    if _NC is None:
        _NC = build_nc()
    return _NC


def _in_maps(p3, p4, p5, boxes, labels, valid, anchors):
    cst = _consts()
    maps = []
    for c in range(N_CORES):
        s = slice(c * B_LOC, (c + 1) * B_LOC)
        pkm = np.empty((P, NPACK), np.float32)
        pkm[:, 0:4] = boxes[s].reshape(P, 4)
        pkm[:, 4] = labels[s].reshape(P)
        pkm[:, 5] = valid[s].reshape(P)
        pkm[:, 6:24] = np.asarray(anchors, np.float32).reshape(1, 18)
        maps.append({
            "p3": np.ascontiguousarray(p3[s]),
            "p4": np.ascontiguousarray(p4[s]),
            "p5": np.ascontiguousarray(p5[s]),
            "pk": pkm,
            "cst": cst,
        })
    return maps


def _combine(partials):
    """Host-side unshard: global sums -> final scalar (mirrors reference)."""
    p = np.sum(np.stack(partials, 0), axis=0, dtype=np.float64)
    cls_t = obj_t = box_t = 0.0
    for li in range(3):
        cls_n, box_n, npos = p[li], p[3 + li], p[6 + li]
        s2 = p[9 + li]
        s1 = 0.0
        for i, (cl, qa, qb, ring) in enumerate(CHUNKS):
            if cl == li:
                cells = PLANE_SHAPES[cl][0] * QUAD * (qb - qa) * N_CORES
                s1 += (0.5 * (p[12 + 3 * i] + p[13 + 3 * i])
                       + p[14 + 3 * i] + LN1P_C[0] * cells)
        denom = max(npos, 1.0)
        n_plane = B_GLOB * 3 * GHWS[li]
        if npos > 0:
            cls_t += cls_n / (denom * NC_CLS)
            obj_t += (s1 - s2) / n_plane
            box_t += box_n / denom
    loss = CLS_GAIN * cls_t + OBJ_GAIN * obj_t + BBOX_GAIN * box_t
    return np.float32(loss)


def _run(inputs, trace=False):
    nc = _get_nc()
    maps = _in_maps(**inputs)
    if trace:
        _install_profile_hook()
    res = run_bass_kernel_spmd(nc, maps, list(range(N_CORES)), trace=trace)
    partials = [np.asarray(res.results[c]["out"], np.float64).sum(axis=0)
                for c in range(N_CORES)]
    return _combine(partials), res


def kernel(p3, p4, p5, boxes, labels, valid, anchors):
    out, _ = _run(dict(p3=p3, p4=p4, p5=p5, boxes=boxes, labels=labels,
                       valid=valid, anchors=anchors))
    return out


# revision 45
# speedup vs baseline: 1.0652x; 1.0652x over previous
"""YOLO-style detection loss on 8 Trainium2 NeuronCores (Bass/Tile).

Data-parallel over batch: each of the 8 cores gets B=2 of the 16 batch
items.  Per core we compute partial sums (per layer: cls numerator,
ciou numerator, npos, dedup'd positive-cell objectness sum s2,
objectness softplus plane sum s1); the host sums partials across cores
(the all-reduce) and applies the global npos normalization.

Perf design (from trace analysis):
- The objectness channel is 1 float every 340B, so a sparse strided
  read is packet-bound: ~20.6ns/packet on 16 DMA engines = 63us/core.
  Instead we read QUADS: one 1024B packet covers 4 obj values (256
  contiguous floats); 12.9MB over 12.6K packets ~= 36-40us, split
  across both hardware-DGE rings (sync + scalar).
- obj_t scatter-max is computed on-chip: cell indices are transposed
  via PE, broadcast via K=1 outer-product matmuls, and duplicate
  (same-cell) positives are masked with a strictly-lower-triangular
  compare; s2 is then a masked dot with the gathered obj values.
  This removes the DRAM scatter/readback round trip.
- All transcendentals use the single Softplus activation table (one
  ACT table load for the whole kernel).
"""
import sys
import types

sys.path.insert(0, "/opt/trn_rl_repo")

import numpy as np

import concourse.bacc as bacc
import concourse.bass as bass
import concourse.mybir as mybir
import concourse.tile as tile
from concourse.bass_utils import run_bass_kernel_spmd
from concourse.tile_rust import add_dep_helper

F32 = mybir.dt.float32
I32 = mybir.dt.int32
OP = mybir.AluOpType
AF = mybir.ActivationFunctionType

N_CORES = 8
B_GLOB = 16
B_LOC = B_GLOB // N_CORES          # 2
M = 64                             # boxes per batch item
P = B_LOC * M                      # 128 partitions = (b, m)
NC_CLS = 80
GWS = [80, 40, 20]                 # grid sizes per layer (square)
GHWS = [g * g for g in GWS]        # 6400, 1600, 400
CELLS = [B_LOC * 3 * g * g for g in GWS]   # 38400, 9600, 2400
# plane layouts: (partitions, cells per partition); cells/part % QUAD == 0
PLANE_SHAPES = [(128, 300), (96, 100), (120, 20)]
QUAD = 4
QL = 85 * (QUAD - 1) + 1           # 256 floats = 1024B per packet
CLS_GAIN, OBJ_GAIN, BBOX_GAIN = 0.5, 1.0, 0.05
IOU_THR = 0.5

# plane DMA chunks: (layer, quad_start, quad_end); all on the sync ring
# (one hw queue saturates the 16 DMA engines; the scalar engine stays
# free for ACT).  Each chunk owns its tiles so its Abs/Exp/poly/reduce
# pipeline starts as soon as its own DMA lands.
CHUNKS = [
    (2, 0, 5, "sync"),
    (1, 0, 20, "scalar"),
    (1, 20, 25, "scalar"),
    (0, 0, 40, "sync"),
    (0, 40, 70, "sync"),
    (0, 70, 75, "sync"),
]
# final tile column layout:
# [cls(3), box(3), npos(3), s2(3)], then per chunk i: sum|x| at 12+3i,
# sum x at 13+3i, sum P(e^-|x|) at 14+3i.  The [P, FINAL_COLS] tile is
# written out unreduced; the host sums partitions and cores.
# softplus = (x+|x|)/2 + ln1p(e^-|x|); ln1p(t) ~= P(t)+C0 on [0,1]
FINAL_COLS = 40
# degree-3 chebyshev fit of ln1p on [0,1]; C0 is added host-side
LN1P_C = (9.25032111e-04, 9.79753413e-01, -3.93535802e-01,
          1.06684733e-01)

# packed consts column layout
C_IOTA, C_SCALES, C_GW1, C_GHW, C_B3, C_TRIL, C_IDENT = (
    0, 80, 83, 86, 89, 90, 218)
NCONST = 346
# packed per-core inputs column layout: bx(4), lab(1), val(1), anc(18)
NPACK = 24


def _install_profile_hook():
    """The agent image's antenv lacks axon_hooks; register it so
    run_bass_kernel_spmd(trace=True) can produce NTFF profiles."""
    if "antenv.axon_hooks" in sys.modules:
        return
    hooks = types.ModuleType("antenv.axon_hooks")
    hooks._hook = None

    def _set(h):
        hooks._hook = h

    def _get():
        return hooks._hook

    hooks.set_axon_ntff_profile_hook = _set
    hooks.get_axon_ntff_profile_hook = _get
    sys.modules["antenv.axon_hooks"] = hooks
    import antenv

    antenv.axon_hooks = hooks
    try:
        from trn_agent_boot.trn_boot import _ntff_profile_via_ctypes

        _set(_ntff_profile_via_ctypes("/opt/axon/libaxon_pjrt.so"))
    except Exception:
        pass


def _consts():
    """Per-core constant input tensor [P, NCONST] (same on every core)."""
    c = np.zeros((P, NCONST), np.float32)
    c[:, C_IOTA:C_IOTA + 80] = np.arange(NC_CLS, dtype=np.float32)
    c[:, C_SCALES:C_SCALES + 3] = np.array(GWS, np.float32)
    c[:, C_GW1:C_GW1 + 3] = np.array([g - 1 for g in GWS], np.float32)
    c[:, C_GHW:C_GHW + 3] = np.array(GHWS, np.float32)
    c[:, C_B3] = 3.0 * (np.arange(P) // M)
    c[:, C_TRIL:C_TRIL + P] = np.tril(np.ones((P, P), np.float32), -1)
    c[:, C_IDENT:C_IDENT + P] = np.eye(P, dtype=np.float32)
    return c


def _ap(a, offset, pattern):
    return bass.AP(tensor=a.tensor, offset=offset, ap=pattern)


def build_nc(stage=99.0):
    nc = bacc.Bacc("TRN2", target_bir_lowering=False)

    pred_ext = [
        nc.dram_tensor("p3", [B_LOC, 3, 80, 80, 85], F32, kind="ExternalInput"),
        nc.dram_tensor("p4", [B_LOC, 3, 40, 40, 85], F32, kind="ExternalInput"),
        nc.dram_tensor("p5", [B_LOC, 3, 20, 20, 85], F32, kind="ExternalInput"),
    ]
    pk_ext = nc.dram_tensor("pk", [P, NPACK], F32, kind="ExternalInput")
    cst_ext = nc.dram_tensor("cst", [P, NCONST], F32, kind="ExternalInput")
    out_ext = nc.dram_tensor("out", [P, FINAL_COLS], F32, kind="ExternalOutput")

    with tile.TileContext(nc) as tc:
        with (
            tc.tile_pool(name="sb", bufs=1) as sb,
            tc.tile_pool(name="ps", bufs=1, space="PSUM") as ps,
        ):
            # ---- plane quad loads: one DMA per chunk; most on the sync
            # ring, a small prefix on scalar (its push finishes before the
            # first ACT work needs the scalar engine) ----
            x2c = []
            for i, (li, qa, qb, ring) in enumerate(CHUNKS):
                prt, cpp = PLANE_SHAPES[li]
                w = qb - qa
                t = sb.tile([prt, w * QL], F32, tag=f"x2c{i}",
                            name=f"x2c{i}")
                eng = nc.sync if ring == "sync" else nc.scalar
                eng.dma_start(
                    t[:],
                    _ap(pred_ext[li][:], 4 + 85 * QUAD * qa,
                        [[85 * cpp, prt], [85 * QUAD, w], [1, QL]]))
                x2c.append(t)

            # ---- packed consts + inputs on the gpsimd (software) ring ----
            cst = sb.tile([P, NCONST], F32, tag="cst", name="cst")
            nc.gpsimd.dma_start(cst[:], cst_ext[:])
            pk = sb.tile([P, NPACK], F32, tag="pk", name="pk")
            nc.gpsimd.dma_start(pk[:], pk_ext[:])

            iota80 = cst[:, C_IOTA:C_IOTA + 80]
            scales = cst[:, C_SCALES:C_SCALES + 3]
            gw1 = cst[:, C_GW1:C_GW1 + 3]
            ghw = cst[:, C_GHW:C_GHW + 3]
            b3 = cst[:, C_B3:C_B3 + 1]
            tril = cst[:, C_TRIL:C_TRIL + P]
            ident = cst[:, C_IDENT:C_IDENT + P]
            bx = pk[:, 0:4]
            lab_f = pk[:, 4:5]
            val_f = pk[:, 5:6]
            anc = pk[:, 6:6 + 18]

            # ---- final accumulator tile ----
            final = sb.tile([P, FINAL_COLS], F32, tag="final", name="final")
            nc.vector.memset(final[:], 0.0)
            ones_r = sb.tile([1, P], F32, tag="ones_r", name="ones_r")
            nc.vector.memset(ones_r[:], 1.0)
            c_cls = final[:, 0:3]
            c_box = final[:, 3:6]
            c_npos = final[:, 6:9]
            c_s2 = final[:, 9:12]

            def t3(tag):
                return sb.tile([P, 3], F32, tag=tag, name=tag)

            tt = nc.vector.tensor_tensor
            ts = nc.vector.tensor_scalar
            stt = nc.vector.scalar_tensor_tensor

            # ---- tbox in grid units: cx,cy,w,h [P,3] (col = layer) ----
            s02, s13, d20, d31 = t3("s02"), t3("s13"), t3("d20"), t3("d31")
            tt(out=s02[:, 0:1], in0=bx[:, 0:1], in1=bx[:, 2:3], op=OP.add)
            tt(out=s13[:, 0:1], in0=bx[:, 1:2], in1=bx[:, 3:4], op=OP.add)
            tt(out=d20[:, 0:1], in0=bx[:, 2:3], in1=bx[:, 0:1], op=OP.subtract)
            tt(out=d31[:, 0:1], in0=bx[:, 3:4], in1=bx[:, 1:2], op=OP.subtract)
            cx, cy, w_, h_ = t3("cx"), t3("cy"), t3("w_"), t3("h_")
            stt(out=cx[:], in0=s02[:, 0:1].to_broadcast([P, 3]), scalar=0.5,
                in1=scales, op0=OP.mult, op1=OP.mult)
            stt(out=cy[:], in0=s13[:, 0:1].to_broadcast([P, 3]), scalar=0.5,
                in1=scales, op0=OP.mult, op1=OP.mult)
            tt(out=w_[:], in0=d20[:, 0:1].to_broadcast([P, 3]), in1=scales,
               op=OP.mult)
            tt(out=h_[:], in0=d31[:, 0:1].to_broadcast([P, 3]), in1=scales,
               op=OP.mult)

            # ---- wh-IoU vs anchors: [P, a(3), l(3)] ----
            def rep_a(ap3):  # [P,3] -> [P,3,3] repeating along anchor dim
                return _ap(ap3, ap3.offset, [ap3.ap[0], [0, 3], [1, 3]])

            # anchor (a,l) views into pk: elem 6 + (l*3+a)*2 (+1 for h)
            pkv = pk[:]
            aw9 = _ap(pkv, pkv.offset + 6, [pkv.ap[0], [2, 3], [6, 3]])
            ah9 = _ap(pkv, pkv.offset + 7, [pkv.ap[0], [2, 3], [6, 3]])

            def t33(tag):
                return sb.tile([P, 3, 3], F32, tag=tag, name=tag)

            m1, m2, inter9, u9 = t33("m1"), t33("m2"), t33("inter9"), t33("u9")
            wh3 = t3("wh3")
            tt(out=m1[:], in0=rep_a(w_[:]), in1=aw9, op=OP.min)
            tt(out=m2[:], in0=rep_a(h_[:]), in1=ah9, op=OP.min)
            tt(out=inter9[:], in0=m1[:], in1=m2[:], op=OP.mult)
            tt(out=wh3[:], in0=w_[:], in1=h_[:], op=OP.mult)
            tt(out=u9[:], in0=aw9, in1=ah9, op=OP.mult)
            tt(out=u9[:], in0=u9[:], in1=rep_a(wh3[:]), op=OP.add)
            tt(out=u9[:], in0=u9[:], in1=inter9[:], op=OP.subtract)
            ts(out=u9[:], in0=u9[:], scalar1=1e-6, scalar2=None, op0=OP.add)
            nc.vector.reciprocal(m1[:], u9[:])
            tt(out=inter9[:], in0=inter9[:], in1=m1[:], op=OP.mult)  # iou

            # argmax over anchors (first-max wins, strict >)
            gt1, gt2, b01 = t3("gt1"), t3("gt2"), t3("b01")
            tt(out=gt1[:], in0=inter9[:, 1, :], in1=inter9[:, 0, :], op=OP.is_gt)
            tt(out=b01[:], in0=inter9[:, 0, :], in1=inter9[:, 1, :], op=OP.max)
            tt(out=gt2[:], in0=inter9[:, 2, :], in1=b01[:], op=OP.is_gt)
            tt(out=b01[:], in0=b01[:], in1=inter9[:, 2, :], op=OP.max)  # best
            # pos mask -> final npos cols
            thr = t3("thr")
            ts(out=thr[:], in0=b01[:], scalar1=IOU_THR, scalar2=None,
               op0=OP.is_gt)
            tt(out=c_npos, in0=thr[:], in1=val_f.to_broadcast([P, 3]),
               op=OP.mult)
            m_all = c_npos  # [P,3] mask, also the npos partials
            # a_f = a01 + gt2*(2 - a01)
            a_f, tmp3 = t3("a_f"), t3("tmp3")
            ts(out=tmp3[:], in0=gt1[:], scalar1=-1.0, scalar2=2.0,
               op0=OP.mult, op1=OP.add)
            tt(out=tmp3[:], in0=tmp3[:], in1=gt2[:], op=OP.mult)
            tt(out=a_f[:], in0=gt1[:], in1=tmp3[:], op=OP.add)

            # ---- grid coords: gx = clip(trunc(cx), 0, gw-1) ----
            gxi = sb.tile([P, 3], I32, tag="gxi", name="gxi")
            gyi = sb.tile([P, 3], I32, tag="gyi", name="gyi")
            gx, gy = t3("gx"), t3("gy")
            corr = t3("corr")
            nc.vector.tensor_copy(gxi[:], cx[:])
            nc.vector.tensor_copy(gx[:], gxi[:])
            tt(out=corr[:], in0=gx[:], in1=cx[:], op=OP.is_gt)
            tt(out=gx[:], in0=gx[:], in1=corr[:], op=OP.subtract)
            nc.vector.tensor_copy(gyi[:], cy[:])
            nc.vector.tensor_copy(gy[:], gyi[:])
            tt(out=corr[:], in0=gy[:], in1=cy[:], op=OP.is_gt)
            tt(out=gy[:], in0=gy[:], in1=corr[:], op=OP.subtract)
            ts(out=gx[:], in0=gx[:], scalar1=0.0, scalar2=None, op0=OP.max)
            tt(out=gx[:], in0=gx[:], in1=gw1, op=OP.min)
            ts(out=gy[:], in0=gy[:], scalar1=0.0, scalar2=None, op0=OP.max)
            tt(out=gy[:], in0=gy[:], in1=gw1, op=OP.min)

            # ---- flat cell index: (3b + a)*ghw + gy*gw + gx ----
            cell = t3("cell")
            stt(out=cell[:], in0=a_f[:], scalar=b3, in1=ghw,
                op0=OP.add, op1=OP.mult)
            tmp_b = t3("tmp_b")
            tt(out=tmp_b[:], in0=gy[:], in1=scales, op=OP.mult)
            tt(out=cell[:], in0=cell[:], in1=tmp_b[:], op=OP.add)
            tt(out=cell[:], in0=cell[:], in1=gx[:], op=OP.add)
            idx = sb.tile([P, 3], I32, tag="idx", name="idx")
            nc.vector.tensor_copy(idx[:], cell[:])

            def poly_ln1p(yt, at, et, last_c0=True):
                # yt/at: ping-pong scratch views, et: t = exp(-|x|)
                # P(t) = ((c3 t + c2) t + c1) t   (c0 added host-side)
                c0, c1, c2, c3 = LN1P_C
                ts(out=at, in0=et, scalar1=c3, scalar2=c2,
                   op0=OP.mult, op1=OP.add)
                tt(out=yt, in0=at, in1=et, op=OP.mult)
                stt(out=at, in0=yt, scalar=c1, in1=et,
                    op0=OP.add, op1=OP.mult)
                if last_c0:
                    ts(out=yt, in0=at, scalar1=c0, scalar2=None, op0=OP.add)
                    return yt
                return at

            def process_chunk(i):
                # sum|x|, sum x, sum P(exp(-|x|)) for plane chunk i
                li, qa, qb, ring = CHUNKS[i]
                prt = PLANE_SHAPES[li][0]
                w = qb - qa
                xv = x2c[i][:]
                sv = _ap(xv, xv.offset, [xv.ap[0], [85, QUAD], [QL, w]])
                at = sb.tile([prt, QUAD, w], F32, tag=f"at{i}", name=f"at{i}")
                et = sb.tile([prt, QUAD, w], F32, tag=f"et{i}", name=f"et{i}")
                yt = sb.tile([prt, QUAD, w], F32, tag=f"yt{i}", name=f"yt{i}")
                nc.scalar.activation(at[:], sv, AF.Abs)
                nc.vector.tensor_reduce(
                    out=final[0:prt, 12 + 3 * i:13 + 3 * i], in_=at[:],
                    axis=mybir.AxisListType.XY, op=OP.add)
                nc.vector.tensor_reduce(
                    out=final[0:prt, 13 + 3 * i:14 + 3 * i], in_=sv,
                    axis=mybir.AxisListType.XY, op=OP.add)
                nc.scalar.activation(et[:], at[:], AF.Exp, scale=-1.0)
                yout = poly_ln1p(yt[:], at[:], et[:], last_c0=False)
                nc.vector.tensor_reduce(
                    out=final[0:prt, 14 + 3 * i:15 + 3 * i], in_=yout,
                    axis=mybir.AxisListType.XY, op=OP.add)

            # first chunks land before the gather-dependent cls work
            process_chunk(0)
            process_chunk(1)
            process_chunk(2)

            # ---- gather pred rows at assigned cells: [P, 3*85] ----
            gath = sb.tile([P, 3 * 85], F32, tag="gath", name="gath")
            for li in range(3):
                flat = pred_ext[li][:].rearrange("b a h w c -> (b a h w) c")
                nc.gpsimd.indirect_dma_start(
                    out=gath[:, 85 * li:85 * (li + 1)],
                    out_offset=None,
                    in_=flat,
                    in_offset=bass.IndirectOffsetOnAxis(
                        ap=idx[:, li:li + 1], axis=0),
                )
            gv = gath[:]

            # ---- cls loss: sum_c softplus(x_c) - x_label, masked ----
            # softplus = (x + |x|)/2 + ln1p(exp(-|x|)); ln1p via a DVE
            # polynomial so the whole kernel uses one ACT table (abs/exp).
            sp_abs, sp_x, sp_ln = t3("sp_abs"), t3("sp_x"), t3("sp_ln")
            xlab = t3("xlab")
            ca240 = sb.tile([P, 3 * NC_CLS], F32, tag="ca240", name="ca240")
            ce240 = sb.tile([P, 3 * NC_CLS], F32, tag="ce240", name="ce240")
            cy240 = sb.tile([P, 3 * NC_CLS], F32, tag="cy240", name="cy240")
            for li in range(3):
                nc.scalar.activation(
                    ca240[:, 80 * li:80 * (li + 1)],
                    gath[:, 85 * li + 5:85 * li + 85], AF.Abs)
                nc.vector.tensor_reduce(
                    out=sp_abs[:, li:li + 1],
                    in_=ca240[:, 80 * li:80 * (li + 1)],
                    axis=mybir.AxisListType.X, op=OP.add)
                nc.vector.tensor_reduce(
                    out=sp_x[:, li:li + 1],
                    in_=gath[:, 85 * li + 5:85 * li + 85],
                    axis=mybir.AxisListType.X, op=OP.add)
            nc.scalar.activation(ce240[:], ca240[:], AF.Exp, scale=-1.0)
            clsy = poly_ln1p(cy240[:], ca240[:], ce240[:], last_c0=True)
            for li in range(3):
                nc.vector.tensor_reduce(
                    out=sp_ln[:, li:li + 1],
                    in_=_ap(clsy, clsy.offset + 80 * li,
                            [clsy.ap[0], [1, 80]]),
                    axis=mybir.AxisListType.X, op=OP.add)
            oh = sb.tile([P, NC_CLS], F32, tag="oh", name="oh")
            stt(out=oh[:], in0=iota80, scalar=lab_f, in1=iota80,
                op0=OP.is_equal, op1=OP.bypass)
            oc = sb.tile([P, NC_CLS], F32, tag="oc", name="oc")
            for li in range(3):
                tt(out=oc[:], in0=oh[:],
                   in1=gath[:, 85 * li + 5:85 * li + 85], op=OP.mult)
                nc.vector.tensor_reduce(
                    out=xlab[:, li:li + 1], in_=oc[:],
                    axis=mybir.AxisListType.X, op=OP.add)
            spsum = sp_abs  # reuse: 0.5*(sum|x| + sum x) + sum ln1p
            tt(out=spsum[:], in0=sp_abs[:], in1=sp_x[:], op=OP.add)
            ts(out=spsum[:], in0=spsum[:], scalar1=0.5, scalar2=None,
               op0=OP.mult)
            tt(out=spsum[:], in0=spsum[:], in1=sp_ln[:], op=OP.add)
            tt(out=spsum[:], in0=spsum[:], in1=xlab[:], op=OP.subtract)
            tt(out=c_cls, in0=spsum[:], in1=m_all, op=OP.mult)

            # ---- CIoU ----
            def gcol(cidx):  # [P,3] view of gathered column cidx
                return _ap(gv, gv.offset + cidx, [gv.ap[0], [85, 3]])

            pcx, pcy, pw, ph = gcol(0), gcol(1), gcol(2), gcol(3)
            pw2, ph2 = t3("pw2"), t3("ph2")
            ts(out=pw2[:], in0=pw, scalar1=0.5, scalar2=None, op0=OP.mult)
            ts(out=ph2[:], in0=ph, scalar1=0.5, scalar2=None, op0=OP.mult)
            px1, px2, py1, py2 = t3("px1"), t3("px2"), t3("py1"), t3("py2")
            tt(out=px1[:], in0=pcx, in1=pw2[:], op=OP.subtract)
            tt(out=px2[:], in0=pcx, in1=pw2[:], op=OP.add)
            tt(out=py1[:], in0=pcy, in1=ph2[:], op=OP.subtract)
            tt(out=py2[:], in0=pcy, in1=ph2[:], op=OP.add)
            tw2, th2 = pw2, ph2  # reuse tiles
            ts(out=tw2[:], in0=w_[:], scalar1=0.5, scalar2=None, op0=OP.mult)
            ts(out=th2[:], in0=h_[:], scalar1=0.5, scalar2=None, op0=OP.mult)
            tx1, tx2, ty1, ty2 = t3("tx1"), t3("tx2"), t3("ty1"), t3("ty2")
            tt(out=tx1[:], in0=cx[:], in1=tw2[:], op=OP.subtract)
            tt(out=tx2[:], in0=cx[:], in1=tw2[:], op=OP.add)
            tt(out=ty1[:], in0=cy[:], in1=th2[:], op=OP.subtract)
            tt(out=ty2[:], in0=cy[:], in1=th2[:], op=OP.add)
            ix1, iy1, ix2, iy2 = t3("ix1"), t3("iy1"), t3("ix2"), t3("iy2")
            tt(out=ix1[:], in0=px1[:], in1=tx1[:], op=OP.max)
            tt(out=iy1[:], in0=py1[:], in1=ty1[:], op=OP.max)
            tt(out=ix2[:], in0=px2[:], in1=tx2[:], op=OP.min)
            tt(out=iy2[:], in0=py2[:], in1=ty2[:], op=OP.min)
            iw, ih = t3("iw"), t3("ih")
            tt(out=iw[:], in0=ix2[:], in1=ix1[:], op=OP.subtract)
            ts(out=iw[:], in0=iw[:], scalar1=0.0, scalar2=None, op0=OP.max)
            tt(out=ih[:], in0=iy2[:], in1=iy1[:], op=OP.subtract)
            ts(out=ih[:], in0=ih[:], scalar1=0.0, scalar2=None, op0=OP.max)
            inter = t3("inter")
            tt(out=inter[:], in0=iw[:], in1=ih[:], op=OP.mult)
            a1, a2, au = t3("a1"), t3("a2"), t3("au")
            tt(out=a1[:], in0=px2[:], in1=px1[:], op=OP.subtract)
            tt(out=au[:], in0=py2[:], in1=py1[:], op=OP.subtract)
            tt(out=a1[:], in0=a1[:], in1=au[:], op=OP.mult)
            tt(out=a2[:], in0=tx2[:], in1=tx1[:], op=OP.subtract)
            tt(out=au[:], in0=ty2[:], in1=ty1[:], op=OP.subtract)
            tt(out=a2[:], in0=a2[:], in1=au[:], op=OP.mult)
            tt(out=au[:], in0=a1[:], in1=a2[:], op=OP.add)
            tt(out=au[:], in0=au[:], in1=inter[:], op=OP.subtract)
            ts(out=au[:], in0=au[:], scalar1=1e-7, scalar2=None, op0=OP.add)
            iou = a1  # reuse
            rcp = t3("rcp")
            nc.vector.reciprocal(rcp[:], au[:])
            tt(out=iou[:], in0=inter[:], in1=rcp[:], op=OP.mult)
            # center distance
            ccx, ccy = t3("ccx"), t3("ccy")
            tt(out=ccx[:], in0=px1[:], in1=px2[:], op=OP.add)
            ts(out=ccx[:], in0=ccx[:], scalar1=0.5, scalar2=None, op0=OP.mult)
            tt(out=ccy[:], in0=tx1[:], in1=tx2[:], op=OP.add)
            ts(out=ccy[:], in0=ccy[:], scalar1=0.5, scalar2=None, op0=OP.mult)
            tt(out=ccx[:], in0=ccx[:], in1=ccy[:], op=OP.subtract)
            tt(out=ccx[:], in0=ccx[:], in1=ccx[:], op=OP.mult)  # dx^2
            cd = a2  # reuse
            tt(out=cd[:], in0=py1[:], in1=py2[:], op=OP.add)
            ts(out=cd[:], in0=cd[:], scalar1=0.5, scalar2=None, op0=OP.mult)
            tt(out=ccy[:], in0=ty1[:], in1=ty2[:], op=OP.add)
            ts(out=ccy[:], in0=ccy[:], scalar1=0.5, scalar2=None, op0=OP.mult)
            tt(out=cd[:], in0=cd[:], in1=ccy[:], op=OP.subtract)
            tt(out=cd[:], in0=cd[:], in1=cd[:], op=OP.mult)     # dy^2
            tt(out=cd[:], in0=ccx[:], in1=cd[:], op=OP.add)
            # enclosing box diag
            ex1, ex2 = t3("ex1"), t3("ex2")
            tt(out=ex1[:], in0=px1[:], in1=tx1[:], op=OP.min)
            tt(out=ex2[:], in0=px2[:], in1=tx2[:], op=OP.max)
            tt(out=ex2[:], in0=ex2[:], in1=ex1[:], op=OP.subtract)
            tt(out=ex2[:], in0=ex2[:], in1=ex2[:], op=OP.mult)  # dx^2
            ey1, ey2 = ix1, ix2  # reuse
            tt(out=ey1[:], in0=py1[:], in1=ty1[:], op=OP.min)
            tt(out=ey2[:], in0=py2[:], in1=ty2[:], op=OP.max)
            tt(out=ey2[:], in0=ey2[:], in1=ey1[:], op=OP.subtract)
            tt(out=ey2[:], in0=ey2[:], in1=ey2[:], op=OP.mult)  # dy^2
            dd = ex2
            tt(out=dd[:], in0=ex2[:], in1=ey2[:], op=OP.add)
            ts(out=dd[:], in0=dd[:], scalar1=1e-7, scalar2=None, op0=OP.add)
            nc.vector.reciprocal(rcp[:], dd[:])
            tt(out=cd[:], in0=cd[:], in1=rcp[:], op=OP.mult)
            tt(out=iou[:], in0=iou[:], in1=cd[:], op=OP.subtract)
            ts(out=iou[:], in0=iou[:], scalar1=-1.0, scalar2=1.0,
               op0=OP.mult, op1=OP.add)                          # ciou loss
            tt(out=c_box, in0=iou[:], in1=m_all, op=OP.mult)

            # ---- s2 via on-chip dedup of same-cell positives ----
            # scatter index: valid -> cell, invalid -> dump cell (=6*ghw)
            cm = t3("cm")
            ts(out=cm[:], in0=ghw, scalar1=6.0, scalar2=None, op0=OP.mult)
            sc = t3("sc")
            tt(out=sc[:], in0=cell[:], in1=cm[:], op=OP.subtract)
            tt(out=sc[:], in0=sc[:], in1=m_all, op=OP.mult)
            tt(out=sc[:], in0=sc[:], in1=cm[:], op=OP.add)
            # transpose each layer's cell column via PE: scT_l[1, P]
            scT_ps = ps.tile([1, 3 * P], F32, tag="scT_ps", name="scT_ps")
            scT = sb.tile([1, 3 * P], F32, tag="scT", name="scT")
            for li in range(3):
                nc.tensor.matmul(out=scT_ps[:, li * P:(li + 1) * P],
                                 lhsT=sc[:, li:li + 1], rhs=ident,
                                 start=True, stop=True)
            nc.vector.tensor_copy(scT[:], scT_ps[:])
            # broadcast each row across partitions via K=1 outer product
            bc_ps = ps.tile([P, 3 * P], F32, tag="bc_ps", name="bc_ps")
            for li in range(3):
                nc.tensor.matmul(out=bc_ps[:, li * P:(li + 1) * P],
                                 lhsT=ones_r[:], rhs=scT[:, li * P:(li + 1) * P],
                                 start=True, stop=True)
            # dup[p] = any earlier same-batch positive with equal cell
            eq = sb.tile([P, P], F32, tag="eq", name="eq")
            dup3 = t3("dup3")
            for li in range(3):
                stt(out=eq[:], in0=bc_ps[:, li * P:(li + 1) * P],
                    scalar=sc[:, li:li + 1], in1=tril,
                    op0=OP.is_equal, op1=OP.mult)
                nc.vector.tensor_reduce(
                    out=dup3[:, li:li + 1], in_=eq[:],
                    axis=mybir.AxisListType.X, op=OP.max)
            keep = t3("keep")
            ts(out=keep[:], in0=dup3[:], scalar1=-1.0, scalar2=1.0,
               op0=OP.mult, op1=OP.add)
            tt(out=keep[:], in0=keep[:], in1=m_all, op=OP.mult)
            tt(out=c_s2, in0=keep[:], in1=gcol(4), op=OP.mult)

            # ---- remaining obj plane chunks (land later in the drain) ----
            for i in range(3, len(CHUNKS)):
                process_chunk(i)

            # ---- write out unreduced; host sums partitions + cores ----
            nc.scalar.dma_start(out_ext[:], final[:])

    nc.finalize()
    return nc


_NC = None


def _get_nc():
    global _NC
    if _NC is None:
        _NC = build_nc()
    return _NC


def _in_maps(p3, p4, p5, boxes, labels, valid, anchors):
    cst = _consts()
    maps = []
    for c in range(N_CORES):
        s = slice(c * B_LOC, (c + 1) * B_LOC)
        pkm = np.empty((P, NPACK), np.float32)
        pkm[:, 0:4] = boxes[s].reshape(P, 4)
        pkm[:, 4] = labels[s].reshape(P)
        pkm[:, 5] = valid[s].reshape(P)
        pkm[:, 6:24] = np.asarray(anchors, np.float32).reshape(1, 18)
        maps.append({
            "p3": np.ascontiguousarray(p3[s]),
            "p4": np.ascontiguousarray(p4[s]),
            "p5": np.ascontiguousarray(p5[s]),
            "pk": pkm,
            "cst": cst,
        })
    return maps


def _combine(partials):
    """Host-side unshard: global sums -> final scalar (mirrors reference)."""
    p = np.sum(np.stack(partials, 0), axis=0, dtype=np.float64)
    cls_t = obj_t = box_t = 0.0
    for li in range(3):
        cls_n, box_n, npos = p[li], p[3 + li], p[6 + li]
        s2 = p[9 + li]
        s1 = 0.0
        for i, (cl, qa, qb, ring) in enumerate(CHUNKS):
            if cl == li:
                cells = PLANE_SHAPES[cl][0] * QUAD * (qb - qa) * N_CORES
                s1 += (0.5 * (p[12 + 3 * i] + p[13 + 3 * i])
                       + p[14 + 3 * i] + LN1P_C[0] * cells)
        denom = max(npos, 1.0)
        n_plane = B_GLOB * 3 * GHWS[li]
        if npos > 0:
            cls_t += cls_n / (denom * NC_CLS)
            obj_t += (s1 - s2) / n_plane
            box_t += box_n / denom
    loss = CLS_GAIN * cls_t + OBJ_GAIN * obj_t + BBOX_GAIN * box_t
    return np.float32(loss)


def _run(inputs, trace=False):
    nc = _get_nc()
    maps = _in_maps(**inputs)
    if trace:
        _install_profile_hook()
    res = run_bass_kernel_spmd(nc, maps, list(range(N_CORES)), trace=trace)
    partials = [np.asarray(res.results[c]["out"], np.float64).sum(axis=0)
                for c in range(N_CORES)]
    return _combine(partials), res


def kernel(p3, p4, p5, boxes, labels, valid, anchors):
    out, _ = _run(dict(p3=p3, p4=p4, p5=p5, boxes=boxes, labels=labels,
                       valid=valid, anchors=anchors))
    return out
